# revision 22
# baseline (speedup 1.0000x reference)
"""Causal multi-head attention with RoPE on 8 Trainium2 NeuronCores.

Problem: x[1,4096,1024], 16 heads, head_dim 64, causal, RoPE(theta=1e4),
Q/K/V/O projections. Sharding: 2 heads per core (head-parallel). Each core:
  - computes Q^T,K^T (transposed, RoPE'd, bf16) and V (natural, bf16) for its
    2 heads
  - flash-style causal attention with scores kept transposed (S^T[k,q]) so
    P^T feeds the P@V matmul directly; softmax denominator comes from an
    appended ones-column in V (M=65 matmul); no max-subtraction needed
    (scores ~ N(0,1) -> exp never overflows)
  - o_proj partial (its 128 ctx columns x full Wo) -> out^T[1024,4096] f32
Host: transposes x / weight slices (cast bf16), builds RoPE cos/sin tables
(f32), sums the 8 partial outputs and transposes back.

Matmul operands are bf16 (1 cyc/row on PE; f32r measured 2 cyc/row);
accumulation is always f32 in PSUM. The softmax-normalization chain
(reciprocal/broadcast) stays f32/f32r.
"""
import os
import sys

sys.path.insert(0, "/opt/trn_rl_repo")

import numpy as np

D_MODEL = 1024
N_HEADS = 16
SEQ = 4096
DHEAD = 64
THETA = 10000.0
N_CORES = 8
CHUNK = 512          # seq chunk = q-block width
NKB = SEQ // 128     # 32 k-blocks of 128


def _build_program():
    from contextlib import ExitStack

    import concourse.bass as bass
    import concourse.mybir as mybir
    import concourse.tile as tile
    from concourse import bacc
    from concourse.masks import make_identity

    F32 = mybir.dt.float32
    F32R = mybir.dt.float32r
    BF16 = mybir.dt.bfloat16
    AF = mybir.ActivationFunctionType

    nc = bacc.Bacc()

    xt_d = nc.dram_tensor("xt", [D_MODEL, SEQ], BF16, kind="ExternalInput")
    wq_d = nc.dram_tensor("wq", [D_MODEL, 128], BF16, kind="ExternalInput")
    wk_d = nc.dram_tensor("wk", [D_MODEL, 128], BF16, kind="ExternalInput")
    wv_d = nc.dram_tensor("wv", [D_MODEL, 128], BF16, kind="ExternalInput")
    wo_d = nc.dram_tensor("wo", [128, D_MODEL], BF16, kind="ExternalInput")
    t1_d = nc.dram_tensor("t1", [128, SEQ], F32, kind="ExternalInput")
    t2_d = nc.dram_tensor("t2", [128, SEQ], F32, kind="ExternalInput")
    m1_d = nc.dram_tensor("m1", [128, 128], BF16, kind="ExternalInput")
    m2_d = nc.dram_tensor("m2", [128, 128], BF16, kind="ExternalInput")
    o_d = nc.dram_tensor("o_part", [D_MODEL, SEQ], F32, kind="ExternalOutput")

    NCHUNK = SEQ // CHUNK  # 8

    with tile.TileContext(nc) as tc:
        with nc.allow_low_precision(reason="bf16 compute; f32 accumulate"), \
             ExitStack() as ctx:
            const = ctx.enter_context(tc.tile_pool(name="const", bufs=1))
            persist = ctx.enter_context(tc.tile_pool(name="persist", bufs=1))
            work = ctx.enter_context(tc.tile_pool(name="work", bufs=1))
            psum = ctx.enter_context(tc.tile_pool(name="psum", bufs=1, space="PSUM"))

            ident = const.tile([128, 128], F32, name="ident", tag="ident")
            make_identity(nc, ident[:])
            ones_raw = const.tile([1, 64], F32, name="ones_raw", tag="ones_raw")
            nc.vector.memset(ones_raw[:], 1.0)
            ones64 = const.tile([1, 64], F32, name="ones64", tag="ones64")
            nc.vector.tensor_copy(ones64[:].bitcast(F32R), ones_raw[:])
            onescol = const.tile([128, 2, 1], BF16, name="onescol", tag="onescol")
            nc.vector.memset(onescol[:], 1.0)
            zbias = const.tile([128, 1], F32, name="zbias", tag="zbias")
            nc.vector.memset(zbias[:], 0.0)

            m1_sb = const.tile([128, 128], BF16, name="m1_sb", tag="m1_sb")
            nc.sync.dma_start(m1_sb[:], m1_d[:])
            m2_sb = const.tile([128, 128], BF16, name="m2_sb", tag="m2_sb")
            nc.sync.dma_start(m2_sb[:], m2_d[:])
            wo_sb = const.tile([128, D_MODEL], BF16, name="wo_sb", tag="wo_sb")

            wq_sb = const.tile([128, 8, 128], BF16, name="wq_sb", tag="wq_sb")
            wk_sb = const.tile([128, 8, 128], BF16, name="wk_sb", tag="wk_sb")
            wv_sb = const.tile([128, 8, 128], BF16, name="wv_sb", tag="wv_sb")
            w_sb = {"q": wq_sb, "k": wk_sb, "v": wv_sb}

            def load_w(d_t, sb):
                # d_t [1024, 128] viewed as [p 128, i 8, col 128]
                nc.sync.dma_start(
                    sb[:], d_t.rearrange("(i p) c -> p i c", i=8))

            load_w(wq_d, wq_sb)

            qt = [None] * NCHUNK   # Q^T chunks [128, 512] bf16 (RoPE'd, d-perm)
            kt = [None] * NCHUNK
            vsb = [None] * NKB     # V natural per k-block [128, 2, 65] bf16
            xts_c = {}
            tabs_c = {}

            def load_chunk(c):
                cs = slice(c * CHUNK, (c + 1) * CHUNK)
                t = work.tile([128, 8, CHUNK], BF16, name=f"xt_{c}",
                              tag="xt", bufs=3)
                nc.sync.dma_start(
                    t[:], xt_d.rearrange("(i p) s -> p i s", i=8)[:, :, cs])
                xts_c[c] = t
                t1c = const.tile([128, CHUNK], F32, name=f"t1c{c}", tag=f"t1c{c}")
                nc.sync.dma_start(t1c[:], t1_d[:, cs])
                t2c = const.tile([128, CHUNK], F32, name=f"t2c{c}", tag=f"t2c{c}")
                nc.sync.dma_start(t2c[:], t2_d[:, cs])
                tabs_c[c] = (t1c, t2c)

            def proj(kind, c):
                w = w_sb[kind]
                xts = xts_c[c]
                ps = psum.tile([128, CHUNK], F32, name=f"{kind}ps{c}",
                               tag="misc", bufs=2)
                for i in range(8):
                    nc.tensor.matmul(ps[:], w[:, i, :], xts[:, i, :],
                                     start=(i == 0), stop=(i == 7))
                if kind in ("q", "k"):
                    t1c, t2c = tabs_c[c]
                    p1 = work.tile([128, CHUNK], BF16, name=f"p1_{kind}{c}",
                                   tag="p1", bufs=3)
                    nc.vector.tensor_mul(p1[:], t1c[:], ps[:])
                    p2 = work.tile([128, CHUNK], BF16, name=f"p2_{kind}{c}",
                                   tag="p2", bufs=3)
                    nc.vector.tensor_mul(p2[:], t2c[:], ps[:])
                    rp = psum.tile([128, CHUNK], F32, name=f"rp_{kind}{c}",
                                   tag="misc", bufs=2)
                    nc.tensor.matmul(rp[:], m1_sb[:], p1[:],
                                     start=True, stop=False)
                    nc.tensor.matmul(rp[:], m2_sb[:], p2[:],
                                     start=False, stop=True)
                    dst = persist.tile([128, CHUNK], BF16,
                                       name=f"{kind}t{c}", tag=f"{kind}t{c}")
                    nc.vector.tensor_copy(dst[:], rp[:])
                    if kind == "q":
                        qt[c] = dst
                    else:
                        kt[c] = dst
                else:
                    vt = work.tile([128, CHUNK], F32, name=f"vt{c}",
                                   tag="vt", bufs=2)
                    nc.vector.tensor_copy(vt[:], ps[:])
                    vt_c[c] = vt

            vt_c = {}

            def vtrans(c, j):
                kb = c * 4 + j
                vn = psum.tile([128, 128], F32, name=f"vn{kb}",
                               tag="misc", bufs=2)
                nc.tensor.transpose(vn[:],
                                    vt_c[c][:, j * 128:(j + 1) * 128],
                                    ident[:])
                vb = persist.tile([128, 2, 65], BF16, name=f"v{kb}",
                                  tag=f"v{kb}")
                nc.vector.tensor_copy(
                    vb[:, :, 0:64],
                    vn[:].rearrange("p (h d) -> p h d", h=2))
                nc.vector.tensor_copy(vb[:, :, 64:65], onescol[:])
                vsb[kb] = vb

            # deferred per-q-block state for the pipelined tail
            pend = {}
            ctx_live = {}
            pend_ctx = {}   # qb -> list of (kbs, [ph_h0, ph_h1]) awaiting ctx MMs

            def attn_pair(qb, p0):
                nkb = 4 * (qb + 1)
                if qb not in ctx_live:
                    ctx_live[qb] = [
                        psum.tile([65, CHUNK], F32, name=f"ctx_{qb}_{h}",
                                  tag="ctx", bufs=2)
                        for h in (0, 1)]
                    pend_ctx[qb] = []
                if True:
                    kbs = list(range(p0, min(p0 + 2, nkb)))
                    s2s = []
                    for h in (0, 1):
                        s2 = psum.tile([128, len(kbs) * 512], F32,
                                       name=f"s2_{qb}_{p0}_{h}", tag="scores",
                                       bufs=2)
                        s2s.append(s2)
                    for j, kb in enumerate(kbs):
                        for h in (0, 1):
                            nc.tensor.matmul(
                                s2s[h][:, j * 512:(j + 1) * 512],
                                kt[kb // 4][h * 64:(h + 1) * 64,
                                            (kb % 4) * 128:(kb % 4) * 128 + 128],
                                qt[qb][h * 64:(h + 1) * 64, :],
                                start=True, stop=True,
                                tile_position=(h * 64, 0))
                    phs = []
                    for h in (0, 1):
                        ph = work.tile([128, len(kbs) * 512], BF16,
                                       name=f"ph_{qb}_{p0}_{h}", tag="p2h", bufs=8)
                        nc.scalar.activation(ph[:], s2s[h][:], AF.Exp,
                                             bias=zbias[:], scale=0.125)
                        # slices of ph per kb; diagonal tiles get a masked copy
                        slices = []
                        for j, kb in enumerate(kbs):
                            if kb >= 4 * qb:  # diagonal: zero k_global > q_global
                                pm = work.tile([128, 512], BF16,
                                               name=f"pm_{qb}_{p0}_{h}_{j}",
                                               tag="phm", bufs=6)
                                nc.gpsimd.affine_select(
                                    out=pm[:],
                                    in_=ph[:, j * 512:(j + 1) * 512],
                                    pattern=[[1, 512]],
                                    compare_op=mybir.AluOpType.is_ge,
                                    fill=0.0,
                                    base=-(kb - 4 * qb) * 128,
                                    channel_multiplier=-1)
                                slices.append((pm, (0, 512)))
                            else:
                                slices.append((ph, (j * 512, (j + 1) * 512)))
                        phs.append(slices)
                    pend_ctx[qb].append((kbs, phs))
                    # emit ctx for the PREVIOUS pending pair (depth-1 pipeline)
                    if len(pend_ctx[qb]) > 1:
                        kbs_prev, phs_prev = pend_ctx[qb].pop(0)
                        _emit_ctx_entry(qb, kbs_prev, phs_prev)

            def _emit_ctx_entry(qb, kbs, phs):
                nkb = 4 * (qb + 1)
                ctx_ps = ctx_live[qb]
                for h in (0, 1):
                    for j, kb in enumerate(kbs):
                        tile_, (lo, hi) = phs[h][j]
                        nc.tensor.matmul(
                            ctx_ps[h][:],
                            vsb[kb][:, h, :],
                            tile_[:, lo:hi],
                            start=(kb == 0), stop=(kb == nkb - 1))

            def attn_finish(qb):
                # flush remaining pending ctx pairs, evacuate ctx psum to SBUF
                # (frees the psum slots for the next q-block immediately), then
                # start the normalization latency chain off-critical-path.
                for kbs_prev, phs_prev in pend_ctx.pop(qb):
                    _emit_ctx_entry(qb, kbs_prev, phs_prev)
                ctx_ps = ctx_live.pop(qb)
                ctxs = []
                for h in (0, 1):
                    cs_ = work.tile([65, CHUNK], F32, name=f"ctxs{qb}{h}",
                                    tag="ctxs", bufs=4)
                    nc.vector.tensor_copy(cs_[:], ctx_ps[h][:])
                    ctxs.append(cs_)
                recs = []
                for h in (0, 1):
                    rec = work.tile([1, CHUNK], F32, name=f"rec{qb}{h}",
                                    tag="rec", bufs=4)
                    nc.vector.reciprocal(rec[:].bitcast(F32R), ctxs[h][64:65, :])
                    recs.append(rec)
                pend[qb] = (ctxs, recs)

            ctxn_live = {}

            def tail_norm(qb):
                ctxs, recs = pend.pop(qb)
                ctxn = work.tile([128, CHUNK], BF16, name=f"ctxn{qb}",
                                 tag="ctxn", bufs=2)
                for h in (0, 1):
                    bc = psum.tile([64, CHUNK], F32, name=f"bc{qb}{h}",
                                   tag="misc", bufs=2)
                    nc.tensor.matmul(bc[:], ones64[:].bitcast(F32R),
                                     recs[h][:].bitcast(F32R),
                                     start=True, stop=True)
                    bcs = work.tile([64, CHUNK], F32, name=f"bcs{qb}{h}",
                                    tag="bcs", bufs=3)
                    nc.vector.tensor_copy(bcs[:], bc[:])
                    nc.vector.tensor_mul(ctxn[h * 64:(h + 1) * 64, :],
                                         ctxs[h][0:64, :], bcs[:])
                ctxn_live[qb] = ctxn

            def tail_oproj(qb, obs):
                ctxn = ctxn_live[qb]
                for ob in obs:
                    o_ps = psum.tile([128, CHUNK], F32, name=f"ops{qb}{ob}",
                                     tag="misc", bufs=2)
                    nc.tensor.matmul(o_ps[:],
                                     wo_sb[:, ob * 128:(ob + 1) * 128],
                                     ctxn[:], start=True, stop=True)
                    osb = work.tile([128, CHUNK], F32, name=f"osb{qb}{ob}",
                                    tag="osb", bufs=3)
                    nc.any.tensor_copy(osb[:], o_ps[:])
                    nc.sync.dma_start(o_d[ob * 128:(ob + 1) * 128,
                                          qb * CHUNK:(qb + 1) * CHUNK], osb[:])

            # prologue
            load_chunk(0)
            proj("q", 0)

            for c in range(NCHUNK):
                npair = 2 * (c + 1)
                # background PE/DMA units for this chunk; units tagged
                # need_early=True must be emitted before the 2 diagonal pairs.
                B = []
                if c == 0:
                    B.append((True, lambda: load_w(wk_d, wk_sb)))
                    B.append((True, lambda: load_w(wv_d, wv_sb)))
                B.append((True, lambda c=c: proj("k", c)))
                B.append((True, lambda c=c: proj("v", c)))
                for j in range(4):
                    B.append((True, lambda c=c, j=j: vtrans(c, j)))
                if c == 1:
                    B.append((False, lambda: nc.sync.dma_start(wo_sb[:], wo_d[:])))
                if c > 0:
                    B.append((False, lambda qb=c - 1: attn_finish(qb)))
                    B.append((False, lambda qb=c - 1: tail_norm(qb)))
                    B.append((False, lambda qb=c - 1: tail_oproj(qb, range(0, 4))))
                    B.append((False, lambda qb=c - 1: tail_oproj(qb, range(4, 8))))
                if c < NCHUNK - 1:
                    B.append((False, lambda c=c: load_chunk(c + 1)))
                    B.append((False, lambda c=c: proj("q", c + 1)))

                # interleave: early units spread over the off-diagonal pairs,
                # late units anywhere; all early units done before pair npair-2.
                early = [u for e, u in B if e]
                late = [u for e, u in B if not e]
                limit = max(npair - 2, 0)
                slots = {}
                for i, u in enumerate(early):
                    pos = (i * limit) // max(len(early), 1)
                    slots.setdefault(min(pos, limit), []).append(u)
                for i, u in enumerate(late):
                    pos = (i * npair) // max(len(late), 1)
                    slots.setdefault(min(pos, npair - 1), []).append(u)
                if limit == 0:
                    for u in slots.pop(0, []):
                        u()
                for p in range(npair):
                    for u in slots.pop(p, []) if limit > 0 or p > 0 else []:
                        u()
                    attn_pair(c, 2 * p)
                for rest in sorted(slots):
                    for u in slots[rest]:
                        u()
            attn_finish(NCHUNK - 1)
            tail_norm(NCHUNK - 1)
            tail_oproj(NCHUNK - 1, range(0, 8))

    nc.compile()
    return nc


_PROG = None


def _get_prog():
    global _PROG
    if _PROG is None:
        _PROG = _build_program()
    return _PROG


def _make_in_maps(inputs):
    import ml_dtypes
    bf16 = ml_dtypes.bfloat16

    x = np.asarray(inputs["x"], dtype=np.float32)
    Wq = np.asarray(inputs["Wq"], dtype=np.float32)
    Wk = np.asarray(inputs["Wk"], dtype=np.float32)
    Wv = np.asarray(inputs["Wv"], dtype=np.float32)
    Wo = np.asarray(inputs["Wo"], dtype=np.float32)
    pos = np.asarray(inputs["token_positions"]).astype(np.float32)

    xt = np.ascontiguousarray(x.reshape(SEQ, D_MODEL).T.astype(bf16))

    ks = np.arange(0, DHEAD, 2, dtype=np.float32)
    inv_freq = (1.0 / np.power(np.float32(THETA), ks / np.float32(DHEAD))).astype(np.float32)
    ang = pos[:, None] * inv_freq[None, :]          # [SEQ, 32]
    cosT = np.cos(ang).T.astype(np.float32)         # [32, SEQ]
    sinT = np.sin(ang).T.astype(np.float32)
    t1 = np.ascontiguousarray(np.concatenate([cosT, -sinT, cosT, -sinT], axis=0))
    t2 = np.ascontiguousarray(np.concatenate([sinT, cosT, sinT, cosT], axis=0))

    perm = np.concatenate([np.arange(0, DHEAD, 2), np.arange(1, DHEAD, 2)])

    m1 = np.zeros((128, 128), dtype=np.float32)
    m2 = np.zeros((128, 128), dtype=np.float32)
    for m in range(128):
        if m % 64 < 32:
            m1[m, m] = 1.0
            m1[m + 32, m] = 1.0
        else:
            m2[m - 32, m] = 1.0
            m2[m, m] = 1.0
    m1 = np.ascontiguousarray(m1.astype(bf16))
    m2 = np.ascontiguousarray(m2.astype(bf16))


    in_maps = []
    for c in range(N_CORES):
        rows = np.arange(c * 128, (c + 1) * 128)
        qk_rows = np.concatenate([c * 128 + h * DHEAD + perm for h in (0, 1)])
        in_maps.append({
            "xt": xt,
            "wq": np.ascontiguousarray(Wq[qk_rows, :].T.astype(bf16)),
            "wk": np.ascontiguousarray(Wk[qk_rows, :].T.astype(bf16)),
            "wv": np.ascontiguousarray(Wv[rows, :].T.astype(bf16)),
            "wo": np.ascontiguousarray(Wo[:, rows].T.astype(bf16)),
            "t1": t1,
            "t2": t2,
            "m1": m1,
            "m2": m2,
        })
    return in_maps


def kernel(x, Wq, Wk, Wv, Wo, token_positions):
    nc = _get_prog()
    in_maps = _make_in_maps({"x": x, "Wq": Wq, "Wk": Wk, "Wv": Wv, "Wo": Wo,
                             "token_positions": token_positions})
    from concourse.bass_utils import run_bass_kernel_spmd

    res = run_bass_kernel_spmd(nc, in_maps, core_ids=list(range(N_CORES)))
    acc = res.results[0]["o_part"].astype(np.float32)
    for i in range(1, N_CORES):
        acc = acc + res.results[i]["o_part"]
    return np.ascontiguousarray(acc.T).reshape(1, SEQ, D_MODEL)


# revision 23
# speedup vs baseline: 1.0709x; 1.0709x over previous
"""Causal multi-head attention with RoPE on 8 Trainium2 NeuronCores.

Problem: x[1,4096,1024], 16 heads, head_dim 64, causal, RoPE(theta=1e4),
Q/K/V/O projections. Sharding: 2 heads per core (head-parallel). Each core:
  - computes Q^T,K^T (transposed, RoPE'd, bf16) and V (natural, bf16) for its
    2 heads
  - flash-style causal attention with scores kept transposed (S^T[k,q]) so
    P^T feeds the P@V matmul directly; softmax denominator comes from an
    appended ones-column in V (M=65 matmul); no max-subtraction needed
    (scores ~ N(0,1) -> exp never overflows)
  - o_proj partial (its 128 ctx columns x full Wo) -> out^T[1024,4096] f32
Host: transposes x / weight slices (cast bf16), builds RoPE cos/sin tables
(f32), sums the 8 partial outputs and transposes back.

Matmul operands are bf16 (1 cyc/row on PE; f32r measured 2 cyc/row);
accumulation is always f32 in PSUM. The softmax-normalization chain
(reciprocal/broadcast) stays f32/f32r.
"""
import os
import sys

sys.path.insert(0, "/opt/trn_rl_repo")

import numpy as np

D_MODEL = 1024
N_HEADS = 16
SEQ = 4096
DHEAD = 64
THETA = 10000.0
N_CORES = 8
CHUNK = 512          # seq chunk = q-block width
NKB = SEQ // 128     # 32 k-blocks of 128


def _build_program():
    from contextlib import ExitStack

    import concourse.bass as bass
    import concourse.mybir as mybir
    import concourse.tile as tile
    from concourse import bacc
    from concourse.masks import make_identity

    F32 = mybir.dt.float32
    F32R = mybir.dt.float32r
    BF16 = mybir.dt.bfloat16
    AF = mybir.ActivationFunctionType

    nc = bacc.Bacc()

    xt_d = nc.dram_tensor("xt", [D_MODEL, SEQ], BF16, kind="ExternalInput")
    wq_d = nc.dram_tensor("wq", [D_MODEL, 128], BF16, kind="ExternalInput")
    wk_d = nc.dram_tensor("wk", [D_MODEL, 128], BF16, kind="ExternalInput")
    wv_d = nc.dram_tensor("wv", [D_MODEL, 128], BF16, kind="ExternalInput")
    wo_d = nc.dram_tensor("wo", [128, D_MODEL], BF16, kind="ExternalInput")
    t1_d = nc.dram_tensor("t1", [128, SEQ], F32, kind="ExternalInput")
    t2_d = nc.dram_tensor("t2", [128, SEQ], F32, kind="ExternalInput")
    m1_d = nc.dram_tensor("m1", [128, 128], BF16, kind="ExternalInput")
    m2_d = nc.dram_tensor("m2", [128, 128], BF16, kind="ExternalInput")
    o_d = nc.dram_tensor("o_part", [D_MODEL, SEQ], F32, kind="ExternalOutput")

    NCHUNK = SEQ // CHUNK  # 8

    with tile.TileContext(nc) as tc:
        with nc.allow_low_precision(reason="bf16 compute; f32 accumulate"), \
             ExitStack() as ctx:
            const = ctx.enter_context(tc.tile_pool(name="const", bufs=1))
            persist = ctx.enter_context(tc.tile_pool(name="persist", bufs=1))
            work = ctx.enter_context(tc.tile_pool(name="work", bufs=1))
            psum = ctx.enter_context(tc.tile_pool(name="psum", bufs=1, space="PSUM"))

            ident = const.tile([128, 128], F32, name="ident", tag="ident")
            make_identity(nc, ident[:])
            ones_raw = const.tile([1, 64], F32, name="ones_raw", tag="ones_raw")
            nc.vector.memset(ones_raw[:], 1.0)
            ones64 = const.tile([1, 64], F32, name="ones64", tag="ones64")
            nc.vector.tensor_copy(ones64[:].bitcast(F32R), ones_raw[:])
            onescol = const.tile([128, 2, 1], BF16, name="onescol", tag="onescol")
            nc.vector.memset(onescol[:], 1.0)
            zbias = const.tile([128, 1], F32, name="zbias", tag="zbias")
            nc.vector.memset(zbias[:], 0.0)

            m1_sb = const.tile([128, 128], BF16, name="m1_sb", tag="m1_sb")
            nc.sync.dma_start(m1_sb[:], m1_d[:])
            # PE warm-up: dense dummy matmuls during the DMA-bound head keep
            # the HAM clock-gate at full rate before real work arrives.
            warm_ps = psum.tile([128, 128], F32, name="warm_ps", tag="misc",
                                bufs=2)
            for _ in range(80):
                nc.tensor.matmul(warm_ps[:], m1_sb[:], m1_sb[:],
                                 start=True, stop=True)
            m2_sb = const.tile([128, 128], BF16, name="m2_sb", tag="m2_sb")
            nc.sync.dma_start(m2_sb[:], m2_d[:])
            wo_sb = const.tile([128, D_MODEL], BF16, name="wo_sb", tag="wo_sb")

            wq_sb = const.tile([128, 8, 128], BF16, name="wq_sb", tag="wq_sb")
            wk_sb = const.tile([128, 8, 128], BF16, name="wk_sb", tag="wk_sb")
            wv_sb = const.tile([128, 8, 128], BF16, name="wv_sb", tag="wv_sb")
            w_sb = {"q": wq_sb, "k": wk_sb, "v": wv_sb}

            def load_w(d_t, sb):
                # d_t [1024, 128] viewed as [p 128, i 8, col 128]
                nc.sync.dma_start(
                    sb[:], d_t.rearrange("(i p) c -> p i c", i=8))

            load_w(wq_d, wq_sb)

            qt = [None] * NCHUNK   # Q^T chunks [128, 512] bf16 (RoPE'd, d-perm)
            kt = [None] * NCHUNK
            vsb = [None] * NKB     # V natural per k-block [128, 2, 65] bf16
            xts_c = {}
            tabs_c = {}

            def load_chunk(c):
                cs = slice(c * CHUNK, (c + 1) * CHUNK)
                t = work.tile([128, 8, CHUNK], BF16, name=f"xt_{c}",
                              tag="xt", bufs=3)
                nc.sync.dma_start(
                    t[:], xt_d.rearrange("(i p) s -> p i s", i=8)[:, :, cs])
                xts_c[c] = t
                t1c = const.tile([128, CHUNK], F32, name=f"t1c{c}", tag=f"t1c{c}")
                nc.sync.dma_start(t1c[:], t1_d[:, cs])
                t2c = const.tile([128, CHUNK], F32, name=f"t2c{c}", tag=f"t2c{c}")
                nc.sync.dma_start(t2c[:], t2_d[:, cs])
                tabs_c[c] = (t1c, t2c)

            def proj(kind, c):
                w = w_sb[kind]
                xts = xts_c[c]
                ps = psum.tile([128, CHUNK], F32, name=f"{kind}ps{c}",
                               tag="misc", bufs=2)
                for i in range(8):
                    nc.tensor.matmul(ps[:], w[:, i, :], xts[:, i, :],
                                     start=(i == 0), stop=(i == 7))
                if kind in ("q", "k"):
                    t1c, t2c = tabs_c[c]
                    p1 = work.tile([128, CHUNK], BF16, name=f"p1_{kind}{c}",
                                   tag="p1", bufs=3)
                    nc.vector.tensor_mul(p1[:], t1c[:], ps[:])
                    p2 = work.tile([128, CHUNK], BF16, name=f"p2_{kind}{c}",
                                   tag="p2", bufs=3)
                    nc.vector.tensor_mul(p2[:], t2c[:], ps[:])
                    rp = psum.tile([128, CHUNK], F32, name=f"rp_{kind}{c}",
                                   tag="misc", bufs=2)
                    nc.tensor.matmul(rp[:], m1_sb[:], p1[:],
                                     start=True, stop=False)
                    nc.tensor.matmul(rp[:], m2_sb[:], p2[:],
                                     start=False, stop=True)
                    dst = persist.tile([128, CHUNK], BF16,
                                       name=f"{kind}t{c}", tag=f"{kind}t{c}")
                    nc.vector.tensor_copy(dst[:], rp[:])
                    if kind == "q":
                        qt[c] = dst
                    else:
                        kt[c] = dst
                else:
                    vt = work.tile([128, CHUNK], F32, name=f"vt{c}",
                                   tag="vt", bufs=2)
                    nc.vector.tensor_copy(vt[:], ps[:])
                    vt_c[c] = vt

            vt_c = {}

            def vtrans(c, j):
                kb = c * 4 + j
                vn = psum.tile([128, 128], F32, name=f"vn{kb}",
                               tag="misc", bufs=2)
                nc.tensor.transpose(vn[:],
                                    vt_c[c][:, j * 128:(j + 1) * 128],
                                    ident[:])
                vb = persist.tile([128, 2, 65], BF16, name=f"v{kb}",
                                  tag=f"v{kb}")
                nc.vector.tensor_copy(
                    vb[:, :, 0:64],
                    vn[:].rearrange("p (h d) -> p h d", h=2))
                nc.vector.tensor_copy(vb[:, :, 64:65], onescol[:])
                vsb[kb] = vb

            # deferred per-q-block state for the pipelined tail
            pend = {}
            ctx_live = {}
            pend_ctx = {}   # qb -> list of (kbs, [ph_h0, ph_h1]) awaiting ctx MMs

            def attn_pair(qb, p0):
                nkb = 4 * (qb + 1)
                if qb not in ctx_live:
                    ctx_live[qb] = [
                        psum.tile([65, CHUNK], F32, name=f"ctx_{qb}_{h}",
                                  tag="ctx", bufs=2)
                        for h in (0, 1)]
                    pend_ctx[qb] = []
                if True:
                    kbs = list(range(p0, min(p0 + 2, nkb)))
                    s2s = []
                    for h in (0, 1):
                        s2 = psum.tile([128, len(kbs) * 512], F32,
                                       name=f"s2_{qb}_{p0}_{h}", tag="scores",
                                       bufs=2)
                        s2s.append(s2)
                    for j, kb in enumerate(kbs):
                        for h in (0, 1):
                            nc.tensor.matmul(
                                s2s[h][:, j * 512:(j + 1) * 512],
                                kt[kb // 4][h * 64:(h + 1) * 64,
                                            (kb % 4) * 128:(kb % 4) * 128 + 128],
                                qt[qb][h * 64:(h + 1) * 64, :],
                                start=True, stop=True,
                                tile_position=(h * 64, 0))
                    phs = []
                    for h in (0, 1):
                        ph = work.tile([128, len(kbs) * 512], BF16,
                                       name=f"ph_{qb}_{p0}_{h}", tag="p2h", bufs=8)
                        nc.scalar.activation(ph[:], s2s[h][:], AF.Exp,
                                             bias=zbias[:], scale=0.125)
                        # slices of ph per kb; diagonal tiles get a masked copy
                        slices = []
                        for j, kb in enumerate(kbs):
                            if kb >= 4 * qb:  # diagonal: zero k_global > q_global
                                pm = work.tile([128, 512], BF16,
                                               name=f"pm_{qb}_{p0}_{h}_{j}",
                                               tag="phm", bufs=6)
                                nc.gpsimd.affine_select(
                                    out=pm[:],
                                    in_=ph[:, j * 512:(j + 1) * 512],
                                    pattern=[[1, 512]],
                                    compare_op=mybir.AluOpType.is_ge,
                                    fill=0.0,
                                    base=-(kb - 4 * qb) * 128,
                                    channel_multiplier=-1)
                                slices.append((pm, (0, 512)))
                            else:
                                slices.append((ph, (j * 512, (j + 1) * 512)))
                        phs.append(slices)
                    pend_ctx[qb].append((kbs, phs))
                    # emit ctx for the PREVIOUS pending pair (depth-1 pipeline)
                    if len(pend_ctx[qb]) > 1:
                        kbs_prev, phs_prev = pend_ctx[qb].pop(0)
                        _emit_ctx_entry(qb, kbs_prev, phs_prev)

            def _emit_ctx_entry(qb, kbs, phs):
                nkb = 4 * (qb + 1)
                ctx_ps = ctx_live[qb]
                for h in (0, 1):
                    for j, kb in enumerate(kbs):
                        tile_, (lo, hi) = phs[h][j]
                        nc.tensor.matmul(
                            ctx_ps[h][:],
                            vsb[kb][:, h, :],
                            tile_[:, lo:hi],
                            start=(kb == 0), stop=(kb == nkb - 1))

            def attn_finish(qb):
                # flush remaining pending ctx pairs, evacuate ctx psum to SBUF
                # (frees the psum slots for the next q-block immediately), then
                # start the normalization latency chain off-critical-path.
                for kbs_prev, phs_prev in pend_ctx.pop(qb):
                    _emit_ctx_entry(qb, kbs_prev, phs_prev)
                ctx_ps = ctx_live.pop(qb)
                ctxs = []
                for h in (0, 1):
                    cs_ = work.tile([65, CHUNK], F32, name=f"ctxs{qb}{h}",
                                    tag="ctxs", bufs=4)
                    nc.vector.tensor_copy(cs_[:], ctx_ps[h][:])
                    ctxs.append(cs_)
                recs = []
                for h in (0, 1):
                    rec = work.tile([1, CHUNK], F32, name=f"rec{qb}{h}",
                                    tag="rec", bufs=4)
                    nc.vector.reciprocal(rec[:].bitcast(F32R), ctxs[h][64:65, :])
                    recs.append(rec)
                pend[qb] = (ctxs, recs)

            ctxn_live = {}

            def tail_norm(qb):
                ctxs, recs = pend.pop(qb)
                ctxn = work.tile([128, CHUNK], BF16, name=f"ctxn{qb}",
                                 tag="ctxn", bufs=2)
                for h in (0, 1):
                    bc = psum.tile([64, CHUNK], F32, name=f"bc{qb}{h}",
                                   tag="misc", bufs=2)
                    nc.tensor.matmul(bc[:], ones64[:].bitcast(F32R),
                                     recs[h][:].bitcast(F32R),
                                     start=True, stop=True)
                    bcs = work.tile([64, CHUNK], F32, name=f"bcs{qb}{h}",
                                    tag="bcs", bufs=3)
                    nc.vector.tensor_copy(bcs[:], bc[:])
                    nc.vector.tensor_mul(ctxn[h * 64:(h + 1) * 64, :],
                                         ctxs[h][0:64, :], bcs[:])
                ctxn_live[qb] = ctxn

            def tail_oproj(qb, obs):
                ctxn = ctxn_live[qb]
                for ob in obs:
                    o_ps = psum.tile([128, CHUNK], F32, name=f"ops{qb}{ob}",
                                     tag="misc", bufs=2)
                    nc.tensor.matmul(o_ps[:],
                                     wo_sb[:, ob * 128:(ob + 1) * 128],
                                     ctxn[:], start=True, stop=True)
                    osb = work.tile([128, CHUNK], F32, name=f"osb{qb}{ob}",
                                    tag="osb", bufs=3)
                    nc.any.tensor_copy(osb[:], o_ps[:])
                    nc.sync.dma_start(o_d[ob * 128:(ob + 1) * 128,
                                          qb * CHUNK:(qb + 1) * CHUNK], osb[:])

            # prologue
            load_chunk(0)
            proj("q", 0)

            for c in range(NCHUNK):
                npair = 2 * (c + 1)
                # background PE/DMA units for this chunk; units tagged
                # need_early=True must be emitted before the 2 diagonal pairs.
                B = []
                if c == 0:
                    B.append((True, lambda: load_w(wk_d, wk_sb)))
                    B.append((True, lambda: load_w(wv_d, wv_sb)))
                B.append((True, lambda c=c: proj("k", c)))
                B.append((True, lambda c=c: proj("v", c)))
                for j in range(4):
                    B.append((True, lambda c=c, j=j: vtrans(c, j)))
                if c == 1:
                    B.append((False, lambda: nc.sync.dma_start(wo_sb[:], wo_d[:])))
                if c > 0:
                    B.append((False, lambda qb=c - 1: attn_finish(qb)))
                    B.append((False, lambda qb=c - 1: tail_norm(qb)))
                    B.append((False, lambda qb=c - 1: tail_oproj(qb, range(0, 4))))
                    B.append((False, lambda qb=c - 1: tail_oproj(qb, range(4, 8))))
                if c < NCHUNK - 1:
                    B.append((False, lambda c=c: load_chunk(c + 1)))
                    B.append((False, lambda c=c: proj("q", c + 1)))

                # interleave: early units spread over the off-diagonal pairs,
                # late units anywhere; all early units done before pair npair-2.
                early = [u for e, u in B if e]
                late = [u for e, u in B if not e]
                limit = max(npair - 2, 0)
                slots = {}
                for i, u in enumerate(early):
                    pos = (i * limit) // max(len(early), 1)
                    slots.setdefault(min(pos, limit), []).append(u)
                for i, u in enumerate(late):
                    pos = (i * npair) // max(len(late), 1)
                    slots.setdefault(min(pos, npair - 1), []).append(u)
                if limit == 0:
                    for u in slots.pop(0, []):
                        u()
                for p in range(npair):
                    for u in slots.pop(p, []) if limit > 0 or p > 0 else []:
                        u()
                    attn_pair(c, 2 * p)
                for rest in sorted(slots):
                    for u in slots[rest]:
                        u()
            attn_finish(NCHUNK - 1)
            tail_norm(NCHUNK - 1)
            tail_oproj(NCHUNK - 1, range(0, 8))

    nc.compile()
    return nc


_PROG = None


def _get_prog():
    global _PROG
    if _PROG is None:
        _PROG = _build_program()
    return _PROG


def _make_in_maps(inputs):
    import ml_dtypes
    bf16 = ml_dtypes.bfloat16

    x = np.asarray(inputs["x"], dtype=np.float32)
    Wq = np.asarray(inputs["Wq"], dtype=np.float32)
    Wk = np.asarray(inputs["Wk"], dtype=np.float32)
    Wv = np.asarray(inputs["Wv"], dtype=np.float32)
    Wo = np.asarray(inputs["Wo"], dtype=np.float32)
    pos = np.asarray(inputs["token_positions"]).astype(np.float32)

    xt = np.ascontiguousarray(x.reshape(SEQ, D_MODEL).T.astype(bf16))

    ks = np.arange(0, DHEAD, 2, dtype=np.float32)
    inv_freq = (1.0 / np.power(np.float32(THETA), ks / np.float32(DHEAD))).astype(np.float32)
    ang = pos[:, None] * inv_freq[None, :]          # [SEQ, 32]
    cosT = np.cos(ang).T.astype(np.float32)         # [32, SEQ]
    sinT = np.sin(ang).T.astype(np.float32)
    t1 = np.ascontiguousarray(np.concatenate([cosT, -sinT, cosT, -sinT], axis=0))
    t2 = np.ascontiguousarray(np.concatenate([sinT, cosT, sinT, cosT], axis=0))

    perm = np.concatenate([np.arange(0, DHEAD, 2), np.arange(1, DHEAD, 2)])

    m1 = np.zeros((128, 128), dtype=np.float32)
    m2 = np.zeros((128, 128), dtype=np.float32)
    for m in range(128):
        if m % 64 < 32:
            m1[m, m] = 1.0
            m1[m + 32, m] = 1.0
        else:
            m2[m - 32, m] = 1.0
            m2[m, m] = 1.0
    m1 = np.ascontiguousarray(m1.astype(bf16))
    m2 = np.ascontiguousarray(m2.astype(bf16))


    in_maps = []
    for c in range(N_CORES):
        rows = np.arange(c * 128, (c + 1) * 128)
        qk_rows = np.concatenate([c * 128 + h * DHEAD + perm for h in (0, 1)])
        in_maps.append({
            "xt": xt,
            "wq": np.ascontiguousarray(Wq[qk_rows, :].T.astype(bf16)),
            "wk": np.ascontiguousarray(Wk[qk_rows, :].T.astype(bf16)),
            "wv": np.ascontiguousarray(Wv[rows, :].T.astype(bf16)),
            "wo": np.ascontiguousarray(Wo[:, rows].T.astype(bf16)),
            "t1": t1,
            "t2": t2,
            "m1": m1,
            "m2": m2,
        })
    return in_maps


def kernel(x, Wq, Wk, Wv, Wo, token_positions):
    nc = _get_prog()
    in_maps = _make_in_maps({"x": x, "Wq": Wq, "Wk": Wk, "Wv": Wv, "Wo": Wo,
                             "token_positions": token_positions})
    from concourse.bass_utils import run_bass_kernel_spmd

    res = run_bass_kernel_spmd(nc, in_maps, core_ids=list(range(N_CORES)))
    acc = res.results[0]["o_part"].astype(np.float32)
    for i in range(1, N_CORES):
        acc = acc + res.results[i]["o_part"]
    return np.ascontiguousarray(acc.T).reshape(1, SEQ, D_MODEL)
